# revision 1
# baseline (speedup 1.0000x reference)
"""Trainium2 Bass kernel for nn_BlockPiecewiseLinear (histogram_binning).

Math reformulation (validated vs the JAX reference to ~6e-7 rel):
    S    = softplus(slope)                      # [.., K+1]
    xs   = sort(x_pos, axis=-1)                 # [.., K]
    dS_j = S[j] - S[j-1]            (j = 1..K, stored at 0..K-1)
    step_j = 1[xs[j-1] <= q]
    A    = sum_j step_j * dS_j
    W    = sum_j step_j * dS_j * xs[j-1]
    slope_sel = (S[0]+EPS) + A
    out  = q*slope_sel - xs[0]*(S[0]+EPS) + xs[0] - W + y_bias

Sharding: pure data-parallel over the batch dim across 8 NeuronCores.
Per-core layout: rows (b,f) flattened; each SBUF tile is [128 part, G rows, K knots].
The 32-knot sort is an alternating-direction bitonic network (15 layers, 50
tensor_tensor min/max ops on AP-regular index subsets); everything else is
elementwise + free-dim reduces on DVE, softplus on ScalarE.
"""

import numpy as np

import concourse.bass as bass
import concourse.bacc as bacc
import concourse.mybir as mybir
import concourse.tile as tile
from concourse.bass_utils import run_bass_kernel_spmd

F32 = mybir.dt.float32
Alu = mybir.AluOpType
Act = mybir.ActivationFunctionType
AxX = mybir.AxisListType.X

B, F, K = 4096, 512, 32
KP1 = K + 1
EPS = 1e-3
NCORES = 8
P = 128
G = 128  # rows per partition per tile; P*G rows/tile


def _bitonic_layers(n=32):
    layers = []
    k = 2
    while k <= n:
        j = k // 2
        while j >= 1:
            layers.append((k, j))
            j //= 2
        k *= 2
    return layers  # 15 layers for n=32


def _emit_sort_layer(nc, cur, dst, kk, jj, g):
    """Alternating-direction bitonic layer (block size kk, distance jj).

    Index bits i = c*2k + d*k + m*2j + e*j + r; d selects sort direction.
    walrus lowers DVE operands as TENSOR3D (3 free dims after stride
    coalescing), so the ascending and descending halves must stay separate
    min/max ops: their fused output pattern would need 4 levels.
    """
    if kk < K and jj == kk // 2:
        # first sub-layer of each k-stage: the m dim is unit and g always
        # coalesces with c (32 = 2k * 16/k), so asc+desc fuse into one min
        # and one max op within TENSOR3D's 3-free-dim limit.
        # i = c*2k + d*k + e*j + r; min -> c*2k + d*(k+j) + r,
        # max -> j + c*2k + d*(k-j) + r
        cs = 16 // kk
        base_s = cur[:, :, :]
        base_d = dst[:, :, :]
        rlev = ([[1, jj]] if jj > 1 else [])
        in_ap = [base_s.ap[0], [2 * kk, g * cs], [kk, 2]] + rlev
        in_lo = bass.AP(tensor=base_s.tensor, offset=base_s.offset, ap=in_ap)
        in_hi = bass.AP(tensor=base_s.tensor, offset=base_s.offset + jj, ap=in_ap)
        out_min = bass.AP(tensor=base_d.tensor, offset=base_d.offset,
                          ap=[base_d.ap[0], [2 * kk, g * cs], [kk + jj, 2]] + rlev)
        out_max = bass.AP(tensor=base_d.tensor, offset=base_d.offset + jj,
                          ap=[base_d.ap[0], [2 * kk, g * cs], [kk - jj, 2]] + rlev)
        nc.vector.tensor_tensor(out=out_min, in0=in_lo, in1=in_hi, op=Alu.min)
        nc.vector.tensor_tensor(out=out_max, in0=in_lo, in1=in_hi, op=Alu.max)
    elif kk < K:
        cs = 16 // kk
        ms = kk // (2 * jj)
        vs = cur[:, :, :].rearrange(
            "p g (c d m e r) -> p g c d m e r", c=cs, d=2, m=ms, e=2, r=jj
        )
        vd = dst[:, :, :].rearrange(
            "p g (c d m e r) -> p g c d m e r", c=cs, d=2, m=ms, e=2, r=jj
        )
        a_lo = vs[:, :, :, 0, :, 0, :]
        a_hi = vs[:, :, :, 0, :, 1, :]
        nc.vector.tensor_tensor(out=vd[:, :, :, 0, :, 0, :], in0=a_lo, in1=a_hi, op=Alu.min)
        nc.vector.tensor_tensor(out=vd[:, :, :, 0, :, 1, :], in0=a_lo, in1=a_hi, op=Alu.max)
        d_lo = vs[:, :, :, 1, :, 0, :]
        d_hi = vs[:, :, :, 1, :, 1, :]
        nc.vector.tensor_tensor(out=vd[:, :, :, 1, :, 0, :], in0=d_lo, in1=d_hi, op=Alu.max)
        nc.vector.tensor_tensor(out=vd[:, :, :, 1, :, 1, :], in0=d_lo, in1=d_hi, op=Alu.min)
    else:
        ms = kk // (2 * jj)
        vs = cur[:, :, :].rearrange(
            "p g (m e r) -> p g m e r", m=ms, e=2, r=jj
        )
        vd = dst[:, :, :].rearrange(
            "p g (m e r) -> p g m e r", m=ms, e=2, r=jj
        )
        a_lo = vs[:, :, :, 0, :]
        a_hi = vs[:, :, :, 1, :]
        nc.vector.tensor_tensor(out=vd[:, :, :, 0, :], in0=a_lo, in1=a_hi, op=Alu.min)
        nc.vector.tensor_tensor(out=vd[:, :, :, 1, :], in0=a_lo, in1=a_hi, op=Alu.max)


def build_nc(nloc, g=G):
    rows_per_tile = P * g
    ntiles = nloc // rows_per_tile
    assert ntiles * rows_per_tile == nloc

    nc = bacc.Bacc("TRN2", target_bir_lowering=False, debug=False)
    x_d = nc.declare_dram_parameter("x", [nloc, K], F32, isOutput=False)
    sl_d = nc.declare_dram_parameter("sl", [nloc, KP1], F32, isOutput=False)
    q_d = nc.declare_dram_parameter("q", [nloc], F32, isOutput=False)
    yb_d = nc.declare_dram_parameter("yb", [P, g], F32, isOutput=False)
    out_d = nc.declare_dram_parameter("out", [nloc], F32, isOutput=True)
    ss_d = nc.declare_dram_parameter("ssel", [nloc], F32, isOutput=True)

    xv = x_d[:, :].rearrange("(t p g) k -> t p g k", p=P, g=g)
    slv = sl_d[:, :].rearrange("(t p g) k -> t p g k", p=P, g=g)
    qv = q_d[:].rearrange("(t p g) -> t p g", p=P, g=g)
    outv = out_d[:].rearrange("(t p g) -> t p g", p=P, g=g)
    ssv = ss_d[:].rearrange("(t p g) -> t p g", p=P, g=g)

    layers = _bitonic_layers(K)

    with tile.TileContext(nc) as tc:
        with (
            tc.tile_pool(name="pyb", bufs=1) as pyb,
            tc.tile_pool(name="px", bufs=2) as px,
            tc.tile_pool(name="psort", bufs=2) as psort,
            tc.tile_pool(name="psl", bufs=1) as psl,
            tc.tile_pool(name="pS", bufs=2) as pS,
            tc.tile_pool(name="ptmp", bufs=3) as ptmp,
            tc.tile_pool(name="psm", bufs=4) as psm,
            tc.tile_pool(name="pq", bufs=4) as pq,
            tc.tile_pool(name="pout", bufs=4) as pout,
        ):
            yb_t = pyb.tile([P, g], F32, tag="yb")
            nc.scalar.dma_start(out=yb_t[:, :], in_=yb_d[:, :])

            # stores of tile t-1 are emitted after tile t's softplus, so on
            # the in-order ACT queue exp/ln(t) aren't stuck behind a store
            # that waits on tile t-1's DVE epilogue
            fin = None
            for t in range(ntiles):
                x_t = px.tile([P, g, K], F32, tag="x")
                nc.scalar.dma_start(out=x_t[:, :, :], in_=xv[t])
                sl_t = psl.tile([P, g, KP1], F32, tag="sl")
                nc.scalar.dma_start(out=sl_t[:, :, :], in_=slv[t])
                q_t = pq.tile([P, g], F32, tag="q")
                nc.scalar.dma_start(out=q_t[:, :], in_=qv[t])

                # softplus = ln(1 + exp(x)); exp in-place on the slope tile
                nc.scalar.activation(
                    out=sl_t[:, :, :], in_=sl_t[:, :, :], func=Act.Exp
                )
                S_t = pS.tile([P, g, KP1], F32, tag="S")
                nc.scalar.activation(
                    out=S_t[:, :, :], in_=sl_t[:, :, :], func=Act.Ln, bias=1.0
                )

                # dS early so the ACT pipeline stays decoupled
                dS_t = ptmp.tile([P, g, K], F32, tag="w3")
                nc.vector.tensor_tensor(
                    out=dS_t[:, :, :], in0=S_t[:, :, 1:KP1], in1=S_t[:, :, 0:K],
                    op=Alu.subtract,
                )
                sm = psm.tile([P, g, 8], F32, tag="sm")
                s0p = sm[:, :, 2]
                nc.vector.tensor_scalar_add(s0p, S_t[:, :, 0], EPS)

                if fin is not None:
                    po_u, po_W, po_v, po_ot, po_st, po_o, po_s = fin
                    nc.vector.tensor_tensor(out=po_v, in0=po_u, in1=po_W, op=Alu.subtract)
                    nc.vector.tensor_tensor(out=po_ot[:, :], in0=po_v, in1=yb_t[:, :], op=Alu.add)
                    nc.scalar.dma_start(out=po_o, in_=po_ot[:, :])
                    nc.scalar.dma_start(out=po_s, in_=po_st[:, :])
                    fin = None

                # ---- bitonic sort of the K knots (ascending) ----
                cur = x_t
                for kk, jj in layers:
                    dst = psort.tile([P, g, K], F32, tag="sort")
                    _emit_sort_layer(nc, cur, dst, kk, jj, g)
                    cur = dst
                xs_t = cur  # sorted ascending [P, g, K]

                # ---- knot-dim elementwise + reduces (DVE) ----
                step_t = ptmp.tile([P, g, K], F32, tag="w3")
                xs_full = xs_t[:, :, :]
                q2ap = q_t[:, :]
                qb = bass.AP(
                    tensor=q2ap.tensor,
                    offset=q2ap.offset,
                    ap=[q2ap.ap[0], q2ap.ap[1], [0, K]],
                )
                nc.vector.tensor_tensor(
                    out=step_t[:, :, :], in0=xs_full, in1=qb, op=Alu.is_le
                )
                m_t = ptmp.tile([P, g, K], F32, tag="w3")
                nc.vector.tensor_tensor(
                    out=m_t[:, :, :], in0=dS_t[:, :, :], in1=step_t[:, :, :],
                    op=Alu.mult,
                )
                w_t = ptmp.tile([P, g, K], F32, tag="w3")
                nc.vector.tensor_tensor(
                    out=w_t[:, :, :], in0=m_t[:, :, :], in1=xs_full, op=Alu.mult
                )

                # epilogue ops that don't need W go between the w-mult and
                # the W-reduce so the DVE pipe drain of w is hidden
                A = sm[:, :, 0]
                W = sm[:, :, 1]
                nc.vector.tensor_reduce(out=A, in_=m_t[:, :, :], axis=AxX, op=Alu.add)

                # ---- epilogue on [P, g] (DVE):
                #   ssel = s0p + A
                #   out  = q*ssel - xmin*s0p + xmin - W + yb
                q2 = q_t[:, :]
                xmin = xs_t[:, :, 0]
                u = sm[:, :, 4]
                v = sm[:, :, 5]
                r = sm[:, :, 6]
                out_t = pout.tile([P, g], F32, tag="out")
                ss_t = pout.tile([P, g], F32, tag="ss")
                nc.vector.tensor_tensor(out=ss_t[:, :], in0=s0p, in1=A, op=Alu.add)
                nc.vector.tensor_tensor(out=u, in0=q2, in1=ss_t[:, :], op=Alu.mult)
                nc.vector.tensor_tensor(out=v, in0=xmin, in1=s0p, op=Alu.mult)
                nc.vector.tensor_tensor(out=r, in0=u, in1=v, op=Alu.subtract)
                nc.vector.tensor_tensor(out=u, in0=r, in1=xmin, op=Alu.add)
                nc.vector.tensor_reduce(out=W, in_=w_t[:, :, :], axis=AxX, op=Alu.add)
                # the two W-dependent ops are deferred into the next
                # iteration (after its dS/s0p ops) to hide W's pipe drain
                fin = (u, W, v, out_t, ss_t, outv[t], ssv[t])

            po_u, po_W, po_v, po_ot, po_st, po_o, po_s = fin
            nc.vector.tensor_tensor(out=po_v, in0=po_u, in1=po_W, op=Alu.subtract)
            nc.vector.tensor_tensor(out=po_ot[:, :], in0=po_v, in1=yb_t[:, :], op=Alu.add)
            nc.scalar.dma_start(out=po_o, in_=po_ot[:, :])
            nc.scalar.dma_start(out=po_s, in_=po_st[:, :])
    nc.compile()
    return nc


_NC_CACHE = {}


def _get_nc(nloc, g=G):
    key = (nloc, g)
    if key not in _NC_CACHE:
        _NC_CACHE[key] = build_nc(nloc, g)
    return _NC_CACHE[key]


def kernel(inputs, x_pos, slope, y_bias):
    inputs = np.ascontiguousarray(np.asarray(inputs, dtype=np.float32))
    x_pos = np.ascontiguousarray(np.asarray(x_pos, dtype=np.float32))
    slope = np.ascontiguousarray(np.asarray(slope, dtype=np.float32))
    y_bias = np.ascontiguousarray(np.asarray(y_bias, dtype=np.float32))

    b, f = inputs.shape
    bloc = b // NCORES
    nloc = bloc * f
    nc = _get_nc(nloc)

    # y_bias expanded to the [P, G] per-tile layout: row (p, g) has f = (p*G+g) % F
    yb_exp = np.ascontiguousarray(np.tile(y_bias[:, 0], (P * G) // f).reshape(P, G))

    in_maps = []
    for c in range(NCORES):
        sl_b = slice(c * bloc, (c + 1) * bloc)
        in_maps.append(
            {
                "x": x_pos[sl_b].reshape(nloc, K),
                "sl": slope[sl_b].reshape(nloc, KP1),
                "q": inputs[sl_b].reshape(nloc),
                "yb": yb_exp,
            }
        )

    res = run_bass_kernel_spmd(nc, in_maps, list(range(NCORES)))
    outs = np.concatenate(
        [res.results[c]["out"].reshape(bloc, f) for c in range(NCORES)], axis=0
    )
    ssel = np.concatenate(
        [res.results[c]["ssel"].reshape(bloc, f) for c in range(NCORES)], axis=0
    )
    return outs, ssel



# revision 4
# speedup vs baseline: 1.3874x; 1.3874x over previous
"""Trainium2 Bass kernel for nn_BlockPiecewiseLinear (histogram_binning).

Math (validated vs the JAX reference):
    S     = softplus(slope)                       # [.., K+1]  fp32 (ACT)
    cnt   = sum_j 1[x_j <= q]  (UNSORTED x, fp32) # exact segment index
    ssel  = S[cnt] + EPS   via  sum_j 1[cnt==j] * S_j          (fp32-exact)
    xs    = sort(x)  in fp16 (bitonic, 15 layers, all DVE 2x mode)
    M_j   = 1[cnt >= j]                           # fp16 mask (exact 0/1)
    W     = sum_j M_j * (S_j - S_{j-1}) * xs[j-1] # fp16 chain
    out   = q*ssel - xs[0]*(S_0+EPS) + xs[0] - W + y_bias

Key points vs the previous version:
  - Only W needs the sorted values; it is continuous in near-ties, so the
    sort runs in fp16 (2x DVE throughput; layout [P, K, G] with rows
    innermost makes every bitonic layer unit-stride and 2x-eligible).
  - ssel is discontinuous in ties, so cnt is computed from fp32 unsorted x
    (exactly matching the reference compare) and S is gathered in fp32.
    W uses the same cnt-derived mask so tie-flips cancel in `out`.
  - Epilogue is batched across all tiles ([P, T*G] fp32 ops).
Sharding: pure data-parallel over batch across 8 NeuronCores.
"""

import numpy as np

import concourse.bass as bass
import concourse.bacc as bacc
import concourse.mybir as mybir
import concourse.tile as tile
from concourse.bass_utils import run_bass_kernel_spmd

F32 = mybir.dt.float32
F16 = mybir.dt.float16
Alu = mybir.AluOpType
Act = mybir.ActivationFunctionType

B, F, K = 4096, 512, 32
KP1 = K + 1
EPS = 1e-3
NCORES = 8
P = 128
G = 64  # rows (innermost) per tile per partition


def _bitonic_layers(n=32):
    layers = []
    k = 2
    while k <= n:
        j = k // 2
        while j >= 1:
            layers.append((k, j))
            j //= 2
        k *= 2
    return layers  # 15 layers for n=32


def _ap(t_ap, off_elems, dims):
    """Raw AP on the same tensor as t_ap: partition entry kept, free dims
    replaced by `dims` ([stride, count] pairs, strides in elements)."""
    return bass.AP(tensor=t_ap.tensor, offset=t_ap.offset + off_elems,
                   ap=[t_ap.ap[0]] + dims)


def _emit_sort_layer(nc, cur, dst, kk, jj, g):
    """One bitonic layer in [P, K, G] layout (knot dim middle, rows inner).

    knot index k = c*2kk + d*kk + m*2jj + e*jj + r; d = sort direction.
    Element address = k*G + g; the (r, g) pair coalesces into one contiguous
    run of jj*G elements, so every op is unit-stride innermost (fp16 2x).
    """
    cs = K // (2 * kk) if kk < K else 0
    base_s = cur[:, :, :]
    base_d = dst[:, :, :]
    inner = [1, jj * g]
    if kk < K and jj == kk // 2:
        # first sublayer of a stage: m is unit; fuse asc+desc via output
        # strides (min lands at d*(kk+jj), max at jj + d*(kk-jj), in knots)
        dims_in = [[2 * kk * g, cs], [kk * g, 2], inner]
        in_lo = _ap(base_s, 0, dims_in)
        in_hi = _ap(base_s, jj * g, dims_in)
        out_min = _ap(base_d, 0, [[2 * kk * g, cs], [(kk + jj) * g, 2], inner])
        out_max = _ap(base_d, jj * g, [[2 * kk * g, cs], [(kk - jj) * g, 2], inner])
        nc.vector.tensor_tensor(out=out_min, in0=in_lo, in1=in_hi, op=Alu.min)
        nc.vector.tensor_tensor(out=out_max, in0=in_lo, in1=in_hi, op=Alu.max)
    elif kk == K:
        # last stage: ascending only
        ms = K // (2 * jj)
        dims_in = [[2 * jj * g, ms], inner]
        in_lo = _ap(base_s, 0, dims_in)
        in_hi = _ap(base_s, jj * g, dims_in)
        nc.vector.tensor_tensor(out=_ap(base_d, 0, dims_in), in0=in_lo, in1=in_hi, op=Alu.min)
        nc.vector.tensor_tensor(out=_ap(base_d, jj * g, dims_in), in0=in_lo, in1=in_hi, op=Alu.max)
    elif cs == 1:
        # kk=16 middle sublayers: c vanishes; fuse asc+desc via d-stride trick
        ms = kk // (2 * jj)
        dims_in = [[kk * g, 2], [2 * jj * g, ms], inner]
        in_lo = _ap(base_s, 0, dims_in)
        in_hi = _ap(base_s, jj * g, dims_in)
        out_min = _ap(base_d, 0, [[(kk + jj) * g, 2], [2 * jj * g, ms], inner])
        out_max = _ap(base_d, jj * g, [[(kk - jj) * g, 2], [2 * jj * g, ms], inner])
        nc.vector.tensor_tensor(out=out_min, in0=in_lo, in1=in_hi, op=Alu.min)
        nc.vector.tensor_tensor(out=out_max, in0=in_lo, in1=in_hi, op=Alu.max)
    else:
        # split by direction: 4 ops
        ms = kk // (2 * jj)
        for d in range(2):
            off = d * kk * g
            dims_in = [[2 * kk * g, cs], [2 * jj * g, ms], inner]
            in_lo = _ap(base_s, off, dims_in)
            in_hi = _ap(base_s, off + jj * g, dims_in)
            o_lo = _ap(base_d, off, dims_in)
            o_hi = _ap(base_d, off + jj * g, dims_in)
            if d == 0:
                nc.vector.tensor_tensor(out=o_lo, in0=in_lo, in1=in_hi, op=Alu.min)
                nc.vector.tensor_tensor(out=o_hi, in0=in_lo, in1=in_hi, op=Alu.max)
            else:
                nc.vector.tensor_tensor(out=o_lo, in0=in_lo, in1=in_hi, op=Alu.max)
                nc.vector.tensor_tensor(out=o_hi, in0=in_lo, in1=in_hi, op=Alu.min)


def _bcast(src2d, n, count_mid):
    """[P, G]-style AP broadcast over a middle dim of size n: [P],[0,n],[1,count]"""
    return bass.AP(tensor=src2d.tensor, offset=src2d.offset,
                   ap=[src2d.ap[0], [0, n], src2d.ap[-1]])


def build_nc(nloc, g=G):
    rows_per_tile = P * g
    T = nloc // rows_per_tile
    assert T * rows_per_tile == nloc
    TG = T * g

    nc = bacc.Bacc("TRN2", target_bir_lowering=False, debug=False)
    x_d = nc.declare_dram_parameter("x", [T * P, K * g], F32, isOutput=False)
    sl_d = nc.declare_dram_parameter("sl", [T * P, KP1 * g], F16, isOutput=False)
    q_d = nc.declare_dram_parameter("q", [P, TG], F32, isOutput=False)
    io_d = nc.declare_dram_parameter("iota", [P, KP1 * g], F16, isOutput=False)
    yb_d = nc.declare_dram_parameter("yb", [P, g], F32, isOutput=False)
    out_d = nc.declare_dram_parameter("out", [P, TG], F32, isOutput=True)
    ss_d = nc.declare_dram_parameter("ssel", [P, TG], F32, isOutput=True)

    xv = x_d[:, :].rearrange("(t p) f -> t p f", p=P)
    slv = sl_d[:, :].rearrange("(t p) f -> t p f", p=P)

    layers = _bitonic_layers(K)

    with tile.TileContext(nc) as tc:
        with (
            tc.tile_pool(name="pacc", bufs=1) as pacc,
            tc.tile_pool(name="pxf", bufs=2) as pxf,
            tc.tile_pool(name="pxh", bufs=2) as pxh,
            tc.tile_pool(name="psort", bufs=2) as psort,
            tc.tile_pool(name="psl", bufs=2) as psl,
            tc.tile_pool(name="pS", bufs=2) as pS,
            tc.tile_pool(name="pSh", bufs=2) as pSh,
            tc.tile_pool(name="p16", bufs=2) as p16,
            tc.tile_pool(name="pf32", bufs=1) as pf32,
        ):
            # whole-core accumulators / constants
            q_t = pacc.tile([P, T, g], F32, tag="q")
            nc.scalar.dma_start(out=q_t[:, :, :], in_=q_d[:, :])
            iota_t = pacc.tile([P, KP1, g], F16, tag="iota")
            nc.scalar.dma_start(out=iota_t[:, :, :], in_=io_d[:, :])
            yb_t = pacc.tile([P, g], F32, tag="yb")
            nc.scalar.dma_start(out=yb_t[:, :], in_=yb_d[:, :])
            W_t = pacc.tile([P, T, g], F32, tag="W")
            ss_t = pacc.tile([P, T, g], F32, tag="ss")
            s0_t = pacc.tile([P, T, g], F32, tag="s0")
            x0_t = pacc.tile([P, T, g], F32, tag="x0")
            ob_t = pacc.tile([P, T, g], F32, tag="ob")

            def epilogue(lo, hi):
                n = (hi - lo) * g
                def s(t3):  # [P, lo:hi, :] as 2D-ish slice
                    return t3[:, lo:hi, :]
                # ssel += EPS (in place)
                nc.vector.tensor_scalar_add(s(ss_t), s(ss_t), EPS)
                # u = q*ssel  (into ob)
                nc.vector.tensor_tensor(out=s(ob_t), in0=s(q_t), in1=s(ss_t), op=Alu.mult)
                # v = xs0*S0e (into s0 slot, reuse)
                nc.vector.tensor_tensor(out=s(s0_t), in0=s(x0_t), in1=s(s0_t), op=Alu.mult)
                nc.vector.tensor_tensor(out=s(ob_t), in0=s(ob_t), in1=s(s0_t), op=Alu.subtract)
                nc.vector.tensor_tensor(out=s(ob_t), in0=s(ob_t), in1=s(x0_t), op=Alu.add)
                nc.vector.tensor_tensor(out=s(ob_t), in0=s(ob_t), in1=s(W_t), op=Alu.subtract)
                ybb = bass.AP(tensor=yb_t[:, :].tensor, offset=yb_t[:, :].offset,
                              ap=[yb_t[:, :].ap[0], [0, hi - lo], [1, g]])
                nc.vector.tensor_tensor(out=s(ob_t), in0=s(ob_t), in1=ybb, op=Alu.add)
                nc.scalar.dma_start(out=out_d[:, lo * g:hi * g], in_=ob_t[:, lo:hi, :])
                nc.scalar.dma_start(out=ss_d[:, lo * g:hi * g], in_=ss_t[:, lo:hi, :])

            for t in range(T):
                # ---- DMAs (alternate x over two queues) ----
                x_f = pxf.tile([P, K, g], F32, tag="x")
                if t % 2 == 0:
                    nc.sync.dma_start(out=x_f[:, :, :], in_=xv[t])
                else:
                    nc.gpsimd.dma_start(out=x_f[:, :, :], in_=xv[t])
                sl_t = psl.tile([P, KP1, g], F16, tag="sl")
                nc.scalar.dma_start(out=sl_t[:, :, :], in_=slv[t])

                # ---- ACT: softplus (fp32 out), casts ----
                S_t = pS.tile([P, KP1, g], F32, tag="S")
                nc.scalar.activation(out=sl_t[:, :, :], in_=sl_t[:, :, :], func=Act.Exp)
                nc.scalar.activation(out=S_t[:, :, :], in_=sl_t[:, :, :], func=Act.Ln, bias=1.0)
                Sh_t = pSh.tile([P, KP1, g], F16, tag="Sh")
                nc.scalar.activation(out=Sh_t[:, :, :], in_=S_t[:, :, :], func=Act.Copy)
                xh_t = pxh.tile([P, K, g], F16, tag="xh")
                nc.scalar.activation(out=xh_t[:, :, :], in_=x_f[:, :, :], func=Act.Copy)

                # ---- cnt path (fp32-exact) ----
                stepU = p16.tile([P, K, g], F16, tag="kg_a")
                qb = _bcast(q_t[:, t, :], K, g)
                nc.vector.tensor_tensor(out=stepU[:, :, :], in0=x_f[:, :, :], in1=qb, op=Alu.is_le)
                c1 = p16.tile([P, 16, g], F16, tag="c1")
                nc.vector.tensor_tensor(out=c1[:, :, :], in0=stepU[:, 0:16, :], in1=stepU[:, 16:32, :], op=Alu.add)
                c2 = p16.tile([P, 8, g], F16, tag="c2")
                nc.vector.tensor_tensor(out=c2[:, :, :], in0=c1[:, 0:8, :], in1=c1[:, 8:16, :], op=Alu.add)
                c3 = p16.tile([P, 4, g], F16, tag="c3")
                nc.vector.tensor_tensor(out=c3[:, :, :], in0=c2[:, 0:4, :], in1=c2[:, 4:8, :], op=Alu.add)
                c4 = p16.tile([P, 2, g], F16, tag="c4")
                nc.vector.tensor_tensor(out=c4[:, :, :], in0=c3[:, 0:2, :], in1=c3[:, 2:4, :], op=Alu.add)
                cnt = p16.tile([P, 1, g], F16, tag="cnt")
                nc.vector.tensor_tensor(out=cnt[:, :, :], in0=c4[:, 0:1, :], in1=c4[:, 1:2, :], op=Alu.add)

                cb_kp1 = _bcast(cnt[:, 0, :], KP1, g)
                delta = p16.tile([P, KP1, g], F16, tag="delta")
                nc.vector.tensor_tensor(out=delta[:, :, :], in0=cb_kp1, in1=iota_t[:, :, :], op=Alu.is_equal)
                cb_k = _bcast(cnt[:, 0, :], K, g)
                M_t = p16.tile([P, K, g], F16, tag="kg_b")
                nc.vector.tensor_tensor(out=M_t[:, :, :], in0=cb_k, in1=iota_t[:, 1:KP1, :], op=Alu.is_ge)

                # ---- fp16 bitonic sort ----
                cur = xh_t
                for kk, jj in layers:
                    dst = psort.tile([P, K, g], F16, tag="sort")
                    _emit_sort_layer(nc, cur, dst, kk, jj, g)
                    cur = dst
                xs_t = cur

                # ---- W chain (fp16) ----
                dS = p16.tile([P, K, g], F16, tag="kg_c")
                nc.vector.tensor_tensor(out=dS[:, :, :], in0=Sh_t[:, 1:KP1, :], in1=Sh_t[:, 0:K, :], op=Alu.subtract)
                m_t = p16.tile([P, K, g], F16, tag="kg_a2")
                nc.vector.tensor_tensor(out=m_t[:, :, :], in0=M_t[:, :, :], in1=dS[:, :, :], op=Alu.mult)
                w_t = p16.tile([P, K, g], F16, tag="kg_c2")
                nc.vector.tensor_tensor(out=w_t[:, :, :], in0=m_t[:, :, :], in1=xs_t[:, :, :], op=Alu.mult)
                t1 = p16.tile([P, 16, g], F16, tag="c1b")
                nc.vector.tensor_tensor(out=t1[:, :, :], in0=w_t[:, 0:16, :], in1=w_t[:, 16:32, :], op=Alu.add)
                t2 = p16.tile([P, 8, g], F16, tag="c2b")
                nc.vector.tensor_tensor(out=t2[:, :, :], in0=t1[:, 0:8, :], in1=t1[:, 8:16, :], op=Alu.add)
                t3 = p16.tile([P, 4, g], F16, tag="c3b")
                nc.vector.tensor_tensor(out=t3[:, :, :], in0=t2[:, 0:4, :], in1=t2[:, 4:8, :], op=Alu.add)
                t4 = p16.tile([P, 2, g], F16, tag="c4b")
                nc.vector.tensor_tensor(out=t4[:, :, :], in0=t3[:, 0:2, :], in1=t3[:, 2:4, :], op=Alu.add)
                nc.vector.tensor_tensor(out=W_t[:, t, :], in0=t4[:, 0, :], in1=t4[:, 1, :], op=Alu.add)

                # ---- ssel path (fp32 gather of S at cnt) ----
                sselP = pf32.tile([P, KP1, g], F32, tag="sselP")
                nc.vector.tensor_tensor(out=sselP[:, :, :], in0=delta[:, :, :], in1=S_t[:, :, :], op=Alu.mult)
                s1 = pf32.tile([P, 16, g], F32, tag="s1")
                nc.vector.tensor_tensor(out=s1[:, :, :], in0=sselP[:, 0:16, :], in1=sselP[:, 16:32, :], op=Alu.add)
                s2 = pf32.tile([P, 8, g], F32, tag="s2")
                nc.vector.tensor_tensor(out=s2[:, :, :], in0=s1[:, 0:8, :], in1=s1[:, 8:16, :], op=Alu.add)
                s3 = pf32.tile([P, 4, g], F32, tag="s3")
                nc.vector.tensor_tensor(out=s3[:, :, :], in0=s2[:, 0:4, :], in1=s2[:, 4:8, :], op=Alu.add)
                s4 = pf32.tile([P, 2, g], F32, tag="s4")
                nc.vector.tensor_tensor(out=s4[:, :, :], in0=s3[:, 0:2, :], in1=s3[:, 2:4, :], op=Alu.add)
                s5 = pf32.tile([P, 1, g], F32, tag="s5")
                nc.vector.tensor_tensor(out=s5[:, :, :], in0=s4[:, 0:1, :], in1=s4[:, 1:2, :], op=Alu.add)
                nc.vector.tensor_tensor(out=ss_t[:, t, :], in0=s5[:, 0, :], in1=sselP[:, K, :], op=Alu.add)

                # ---- per-tile epilogue inputs ----
                nc.vector.tensor_scalar_add(s0_t[:, t, :], S_t[:, 0, :], EPS)
                nc.vector.tensor_scalar_add(x0_t[:, t, :], xs_t[:, 0, :], 0.0)

                if t == T // 2:
                    epilogue(0, T // 2)
            epilogue(T // 2, T)
    nc.compile()
    return nc


_NC_CACHE = {}


def _get_nc(nloc, g=G):
    key = (nloc, g)
    if key not in _NC_CACHE:
        _NC_CACHE[key] = build_nc(nloc, g)
    return _NC_CACHE[key]


def prepare_in_maps(inputs, x_pos, slope, y_bias):
    """Shard + relayout full inputs for the 8 cores. Returns (nloc, in_maps)."""
    inputs = np.asarray(inputs, dtype=np.float32)
    x_pos = np.asarray(x_pos, dtype=np.float32)
    slope = np.asarray(slope, dtype=np.float32)
    y_bias = np.asarray(y_bias, dtype=np.float32)
    b, f = inputs.shape
    bloc = b // NCORES
    nloc = bloc * f
    T = nloc // (P * G)

    # constants shared by all cores
    iota = np.arange(KP1, dtype=np.float16)
    iota_full = np.ascontiguousarray(
        np.broadcast_to(iota[None, :, None], (P, KP1, G)).reshape(P, KP1 * G)
    )
    # row r = t*(P*G) + p*G + g  ->  f = r mod F = ((p*G + g) mod F) (P*G*T multiple of F)
    pg = (np.arange(P)[:, None] * G + np.arange(G)[None, :]) % f
    yb_exp = np.ascontiguousarray(y_bias[:, 0][pg])

    in_maps = []
    for c in range(NCORES):
        sl_b = slice(c * bloc, (c + 1) * bloc)
        x = x_pos[sl_b].reshape(T, P, G, K).transpose(0, 1, 3, 2)
        x = np.ascontiguousarray(x).reshape(T * P, K * G)
        sl = slope[sl_b].astype(np.float16).reshape(T, P, G, KP1).transpose(0, 1, 3, 2)
        sl = np.ascontiguousarray(sl).reshape(T * P, KP1 * G)
        q = inputs[sl_b].reshape(T, P, G).transpose(1, 0, 2)
        q = np.ascontiguousarray(q).reshape(P, T * G)
        in_maps.append({"x": x, "sl": sl, "q": q, "iota": iota_full, "yb": yb_exp})
    return nloc, in_maps


def unpack_results(res, b, f):
    bloc = b // NCORES
    nloc = bloc * f
    T = nloc // (P * G)
    outs, ssels = [], []
    for c in range(NCORES):
        o = res.results[c]["out"].reshape(P, T, G).transpose(1, 0, 2).reshape(bloc, f)
        s = res.results[c]["ssel"].reshape(P, T, G).transpose(1, 0, 2).reshape(bloc, f)
        outs.append(o)
        ssels.append(s)
    return np.concatenate(outs, 0), np.concatenate(ssels, 0)


def kernel(inputs, x_pos, slope, y_bias):
    b, f = np.asarray(inputs).shape
    nloc, in_maps = prepare_in_maps(inputs, x_pos, slope, y_bias)
    nc = _get_nc(nloc)
    res = run_bass_kernel_spmd(nc, in_maps, list(range(NCORES)))
    return unpack_results(res, b, f)
